# revision 6
# baseline (speedup 1.0000x reference)
"""Causal self-attention (B=2, T=2048, D=1024, H=16, Dh=64) on 8 TRN2 cores.

Sharding: core c -> batch b = c//4 (data parallel), head group g = c%4
(tensor parallel, 4 heads = 256 dims). Each core computes a full-shape
[T, D] partial of the output projection for its (b, g); the host sums
the 4 head-group partials per batch (bf16 partials, fp32 host sum).

Single fused pipeline, bf16 compute with fp32 PSUM accumulation.
Projections (x-chunk granularity), attention (512-col tq regions, head
pairs row-packed), and the output projection are interleaved in
emission order so the PE stays busy while ScalarE chews exp:

  P(0) | A(0,0) P(1) A(1,0) O(0) | A(0,1) P(2) A(1,1) O(1) | ... | O(3)

PSUM: one shared 2-buf ring of [128,1024] tiles serves projection (pq/
pv), attention scores (sT), and out-proj (po) = 4 banks; oTa/oTb
[65, 512] x 2 bufs = 4 banks.

Softmax normalization per (pair, region, head): DVE reciprocal directly
on the PSUM l-row (row 64 of oT, from the ones-column of v), DMA
partition-broadcast of r to [64, 512], then one fused DVE multiply
PSUM x r -> y_sb bf16 (also the only copy of O out of PSUM).
"""

import numpy as np
from contextlib import ExitStack

import concourse.bass as bass
import concourse.tile as tile
from concourse import bacc, mybir
from concourse.bass_utils import run_bass_kernel_spmd

F32 = mybir.dt.float32
BF16 = mybir.dt.bfloat16
CDT = BF16

B, T, D = 2, 2048, 1024
H_TOT, DH = 16, 64
HL = 4                # local heads per core
DG = HL * DH          # 256 local head dims
NT = T // 128         # 16 t-tiles
NCH = T // 512        # 4 t-chunks / tq regions
CT = D // 128         # 8 c-tiles

_CACHE = {}


def build():
    nc = bacc.Bacc("TRN2", target_bir_lowering=False, debug=False, num_devices=8)
    xT_d = nc.dram_tensor("xT", [NCH, 128, CT, 512], CDT, kind="ExternalInput").ap()
    wq_d = nc.dram_tensor("wq", [128, CT, DG], CDT, kind="ExternalInput").ap()
    wk_d = nc.dram_tensor("wk", [128, CT, DG], CDT, kind="ExternalInput").ap()
    wv_d = nc.dram_tensor("wv", [128, CT, DG], CDT, kind="ExternalInput").ap()
    wo_d = nc.dram_tensor("wo", [128, 2, D], CDT, kind="ExternalInput").ap()
    mask_d = nc.dram_tensor("mask", [128, 2, 128], CDT, kind="ExternalInput").ap()
    out_d = nc.dram_tensor("out", [T, D], CDT, kind="ExternalOutput").ap()

    with tile.TileContext(nc) as tc:
        with ExitStack() as ctx:
            cons = ctx.enter_context(tc.tile_pool(name="cons", bufs=1))
            xp = ctx.enter_context(tc.tile_pool(name="xp", bufs=2))
            pp = ctx.enter_context(tc.tile_pool(name="pp", bufs=4))
            nrm = ctx.enter_context(tc.tile_pool(name="nrm", bufs=4))
            outp = ctx.enter_context(tc.tile_pool(name="outp", bufs=4))
            mm = ctx.enter_context(tc.tile_pool(name="mm", bufs=2, space="PSUM"))
            op_ = ctx.enter_context(tc.tile_pool(name="op", bufs=2, space="PSUM"))

            wq_sb = cons.tile([128, CT, DG], CDT)
            wk_sb = cons.tile([128, CT, DG], CDT)
            wv_sb = cons.tile([128, CT, DG], CDT)
            wo_sb = cons.tile([128, 2, D], CDT)
            mask_sb = cons.tile([128, 2, 128], CDT)

            qsb = cons.tile([128, 2, T], CDT)
            ksb = cons.tile([128, 2, T], CDT)
            v_sb = cons.tile([128, NT, HL, DH + 1], CDT)
            nc.vector.memset(v_sb[:, :, :, DH], 1.0)
            y_sb = cons.tile([128, 2, T], CDT)

            # ---- input prefetch: x chunk 0 wide across queues, weights ----
            def load_x(n):
                x_sb = xp.tile([128, CT, 512], CDT, tag="x", name=f"x{n}")
                engs = (nc.sync, nc.scalar, nc.gpsimd, nc.sync)
                for qi, e in enumerate(engs):
                    e.dma_start(
                        x_sb[:, 2 * qi : 2 * qi + 2, :],
                        xT_d[n, :, 2 * qi : 2 * qi + 2, :],
                    )
                return x_sb

            x_cur = load_x(0)
            nc.sync.dma_start(wq_sb[:], wq_d[:])
            nc.scalar.dma_start(wk_sb[:], wk_d[:])
            nc.gpsimd.dma_start(wv_sb[:], wv_d[:])
            nc.scalar.dma_start(mask_sb[:], mask_d[:])
            nc.sync.dma_start(wo_sb[:], wo_d[:])

            # ---- projection chunk n as a list of per-PSUM-tile emitters ----
            def proj_emitters(n, x_sb):
                ems = []
                for w_sb, dst in ((wq_sb, qsb), (wk_sb, ksb)):
                    for j2 in range(2):
                        def em(w_sb=w_sb, dst=dst, j2=j2):
                            pq = mm.tile([128, 1024], F32, tag="mm",
                                         name=f"pq{n}_{0 if dst is qsb else 1}_{j2}")
                            for ct in range(CT):
                                nc.tensor.matmul(
                                    pq[:, 0:512],
                                    w_sb[:, ct, 128 * j2 : 128 * (j2 + 1)],
                                    x_sb[:, ct, :],
                                    start=(ct == 0),
                                    stop=(ct == CT - 1),
                                )
                            nc.vector.tensor_copy(
                                dst[:, j2, 512 * n : 512 * (n + 1)], pq[:, 0:512]
                            )
                        ems.append(em)
                for i in range(4):
                    def em(i=i):
                        pv = mm.tile([128, 1024], F32, tag="mm", name=f"pv{n}_{i}")
                        for ct in range(CT):
                            nc.tensor.matmul(
                                pv[:, 0:DG],
                                x_sb[:, ct, 128 * i : 128 * (i + 1)],
                                wv_sb[:, ct, :],
                                start=(ct == 0),
                                stop=(ct == CT - 1),
                            )
                        nc.vector.tensor_copy(
                            v_sb[:, 4 * n + i, :, 0:DH],
                            pv[:, 0:DG].rearrange("p (h d) -> p h d", h=HL),
                        )
                    ems.append(em)
                return ems

            # ---- attention for (pair p, region reg); fillers interleave ----
            def attn(p, reg, fillers):
                c0r, c1r = 512 * reg, 512 * (reg + 1)
                jlast = 4 * reg + 3
                oTa = op_.tile([DH + 1, 512], F32, tag="oTa", name=f"oTa{p}_{reg}")
                oTb = op_.tile([DH + 1, 512], F32, tag="oTb", name=f"oTb{p}_{reg}")

                def emit_st(j):
                    c0 = max(c0r, 128 * j)
                    w = c1r - c0
                    sT = mm.tile([128, 1024], F32, tag="mm", name=f"sT{p}_{reg}_{j}")
                    nc.tensor.matmul(
                        sT[:, 0:w],
                        ksb[0:DH, p, 128 * j : 128 * (j + 1)],
                        qsb[0:DH, p, c0:c1r],
                        start=True,
                        stop=True,
                    )
                    nc.tensor.matmul(
                        sT[:, 512 : 512 + w],
                        ksb[DH:128, p, 128 * j : 128 * (j + 1)],
                        qsb[DH:128, p, c0:c1r],
                        start=True,
                        stop=True,
                    )
                    pT = pp.tile([128, 1024], CDT, tag="pT", name=f"pT{p}_{reg}_{j}")
                    sT3 = sT[:].rearrange("p (h c) -> p h c", h=2)
                    pT3 = pT[:].rearrange("p (h c) -> p h c", h=2)
                    nc.scalar.activation(
                        pT3[:, :, 0:w],
                        sT3[:, :, 0:w],
                        mybir.ActivationFunctionType.Exp,
                        scale=0.125,
                    )
                    if j >= 4 * reg:  # diagonal block at rel cols [0,128)
                        nc.vector.tensor_mul(
                            pT3[:, :, 0:128], pT3[:, :, 0:128], mask_sb[:]
                        )
                    return pT

                def emit_pv(j, pT):
                    c0 = max(c0r, 128 * j)
                    w = c1r - c0
                    nc.tensor.matmul(
                        oTa[:, c0 - c0r :],
                        v_sb[:, j, 2 * p, :],
                        pT[:, 0:w],
                        start=(j == 0),
                        stop=(j == jlast),
                        skip_group_check=True,
                    )
                    nc.tensor.matmul(
                        oTb[:, c0 - c0r :],
                        v_sb[:, j, 2 * p + 1, :],
                        pT[:, 512 : 512 + w],
                        start=(j == 0),
                        stop=(j == jlast),
                        skip_group_check=True,
                    )

                nfill = len(fillers)
                njs = jlast + 1
                fi = 0
                prev = None
                for j in range(njs):
                    pT = emit_st(j)
                    if prev is not None:
                        emit_pv(*prev)
                    prev = (j, pT)
                    # fillers spread across the j loop
                    due = (j + 1) * nfill // njs
                    while fi < due:
                        fillers[fi]()
                        fi += 1
                emit_pv(*prev)
                while fi < nfill:
                    fillers[fi]()
                    fi += 1
                return oTa, oTb

            # ---- softmax normalization + y cast for (p, reg) ----
            def norm(p, reg, oTa, oTb):
                c0r, c1r = 512 * reg, 512 * (reg + 1)
                for h, oT in ((2 * p, oTa), (2 * p + 1, oTb)):
                    hp = 64 * (h % 2)
                    rrow = nrm.tile([1, 512], F32, tag="rr", name=f"rr{p}_{reg}_{h}")
                    nc.vector.reciprocal(rrow[:], oT[DH : DH + 1, :])
                    rb = nrm.tile([DH, 512], F32, tag="rb", name=f"rb{p}_{reg}_{h}")
                    nc.gpsimd.partition_broadcast(rb[:], rrow[:])
                    nc.vector.tensor_mul(
                        y_sb[hp : hp + DH, p, c0r:c1r], oT[0:DH, :], rb[:]
                    )

            # ---- output projection region reg ----
            def outproj(reg):
                for i in range(4 * reg, 4 * reg + 4):
                    for oc in range(2):
                        po = mm.tile([128, 1024], F32, tag="mm", name=f"po{i}_{oc}")
                        for g2 in range(2):
                            nc.tensor.matmul(
                                po[:, 0:512],
                                y_sb[:, g2, 128 * i : 128 * (i + 1)],
                                wo_sb[:, g2, 512 * oc : 512 * (oc + 1)],
                                start=(g2 == 0),
                                stop=(g2 == 1),
                            )
                        o_sb = outp.tile([128, 512], CDT, tag="o", name=f"o{i}_{oc}")
                        if (i + oc) % 2 == 0:
                            nc.vector.tensor_copy(o_sb[:], po[:, 0:512])
                        else:
                            nc.scalar.copy(o_sb[:], po[:, 0:512])
                        eng = (nc.gpsimd, nc.sync, nc.scalar)[(2 * i + oc) % 3]
                        eng.dma_start(
                            out_d[
                                128 * i : 128 * (i + 1), 512 * oc : 512 * (oc + 1)
                            ],
                            o_sb[:],
                        )

            # ---- the fused schedule ----
            for em in proj_emitters(0, x_cur):
                em()
            for reg in range(NCH):
                if reg + 1 < NCH:
                    x_next = load_x(reg + 1)
                    fillers = proj_emitters(reg + 1, x_next)
                else:
                    x_next, fillers = None, []
                oTa0, oTb0 = attn(0, reg, fillers)
                norm(0, reg, oTa0, oTb0)
                oTa1, oTb1 = attn(1, reg, [])
                norm(1, reg, oTa1, oTb1)
                outproj(reg)
                x_cur = x_next
    nc.compile()
    return nc


def make_in_maps(x, Wq, Wk, Wv, Wo):
    import ml_dtypes

    cnp = ml_dtypes.bfloat16
    mask1 = np.triu(np.ones((128, 128), dtype=cnp))  # [tk, tq] valid tk<=tq
    mask = np.ascontiguousarray(
        np.broadcast_to(mask1[:, None, :], (128, 2, 128))
    )
    in_maps = []
    for c in range(8):
        b, g = c // 4, c % 4
        rows = slice(DG * g, DG * (g + 1))
        in_maps.append(
            {
                "xT": np.ascontiguousarray(
                    x[b].T.reshape(CT, 128, NCH, 512).transpose(2, 1, 0, 3)
                ).astype(cnp),
                "wq": np.ascontiguousarray(
                    Wq[rows].T.reshape(CT, 128, DG).transpose(1, 0, 2)
                ).astype(cnp),
                "wk": np.ascontiguousarray(
                    Wk[rows].T.reshape(CT, 128, DG).transpose(1, 0, 2)
                ).astype(cnp),
                "wv": np.ascontiguousarray(
                    Wv[rows].T.reshape(CT, 128, DG).transpose(1, 0, 2)
                ).astype(cnp),
                "wo": np.ascontiguousarray(
                    Wo[:, rows].T.reshape(2, 128, D).transpose(1, 0, 2)
                ).astype(cnp),
                "mask": mask,
            }
        )
    return in_maps


def _run(x, Wq, Wk, Wv, Wo, trace=False):
    if "nc" not in _CACHE:
        _CACHE["nc"] = build()
    nc = _CACHE["nc"]
    in_maps = make_in_maps(x, Wq, Wk, Wv, Wo)
    res = run_bass_kernel_spmd(nc, in_maps, core_ids=list(range(8)), trace=trace)
    out = np.zeros((B, T, D), dtype=np.float32)
    for c in range(8):
        out[c // 4] += res.results[c]["out"].astype(np.float32)
    return out, res


def kernel(x, Wq, Wk, Wv, Wo):
    out, _ = _run(
        np.asarray(x, dtype=np.float32),
        np.asarray(Wq, dtype=np.float32),
        np.asarray(Wk, dtype=np.float32),
        np.asarray(Wv, dtype=np.float32),
        np.asarray(Wo, dtype=np.float32),
    )
    return out
